# revision 20
# baseline (speedup 1.0000x reference)
"""Trainium2 Bass kernel for CausalSelfAttention (GQA + alibi, B=2, T=2048,
d_model=2048, 16 q heads / 4 kv heads).

Sharding: 8 cores = (batch b in {0,1}) x (kv-group g in {0..3}).
Each core computes, for its (b, g):
  - QKV^T slice:  [768, T] in fp16 (4 q heads pre-scaled by 1/sqrt(hd),
    1 k head, 1 v head)
  - causal attention for its 4 query heads, scores transposed (keys j on
    partitions).  alibi+mask as fp16 additive tiles on DVE; exp on ACT
    writes fp16 P; softmax denominator via DVE tree-sum of the exp tiles
    followed by a single ones-matmul per (head, i-block); normalization
    broadcast via a K=1 matmul.
  - partial output projection O_slice[t, 512] @ proj_w[:, slice]^T.
Host sums the 4 partials per batch and adds proj_b.

Pipeline: stage-1 (qkv) chains for block tb+1 and stage-3 (proj) chains
for block tb-1 are interleaved into the attention instruction stream of
block tb as PE filler, so the tensor engine never waits on DVE/ACT.
"""

import math

import numpy as np

D = 2048
T = 2048
NH = 16
KVH = 4
HD = 128
GRP = 4
B = 2
NCORE = 8
FB = 6          # qkv feature tiles of 128 (4 q heads + k + v)
NEG16 = -30000.0
AV_LAG = 3      # jb-rounds between score and AV emission

_CACHE: dict = {}


# --------------------------------------------------------------------------
# device kernel
# --------------------------------------------------------------------------

def _build_nc():
    import concourse.mybir as mybir
    from concourse import bacc
    import concourse.tile as tile
    f32 = mybir.dt.float32
    fp16 = mybir.dt.float16
    Exp = mybir.ActivationFunctionType.Exp
    Ident = mybir.ActivationFunctionType.Identity
    add = mybir.AluOpType.add
    mult = mybir.AluOpType.mult

    nc = bacc.Bacc("TRN2", target_bir_lowering=False, debug=False,
                   num_devices=NCORE)

    # xt: x^T, tb-major then dt: xt[p, tb*8192 + dt*512 + tl]
    xt_d = nc.dram_tensor("xt", [128, 16 * T], fp16, kind="ExternalInput").ap()
    # wt: fb-major then dt: wt[p, fb*2048 + dt*128 + m]
    wt_d = nc.dram_tensor("wt", [128, FB * 2048], fp16,
                          kind="ExternalInput").ap()
    # bqcb: cols 0:6 qkv bias (per fb), cols 6:54 lower-block alibi scalars
    bqcb_d = nc.dram_tensor("bqcb", [128, 54], f32, kind="ExternalInput").ap()
    # at: cols 0:2048 atr (4 heads x 512), cols 2048:10240 atd (16 x 512)
    at_d = nc.dram_tensor("at", [128, 20 * 512], fp16,
                          kind="ExternalInput").ap()
    # pt: proj slice, head-major: pt[p, h*2048 + n]
    pt_d = nc.dram_tensor("pt", [128, 4 * T], fp16, kind="ExternalInput").ap()
    # kn: col 0 ones col; [0, 1:129] ones row; [:, 130:258] identity
    kn_d = nc.dram_tensor("kn", [128, 258], fp16, kind="ExternalInput").ap()
    out_d = nc.dram_tensor("out", [T, D], f32, kind="ExternalOutput").ap()

    with tile.TileContext(nc) as tc:
        with tc.tile_pool(name="persist", bufs=1) as pp, \
             tc.tile_pool(name="xtp", bufs=3) as xtp, \
             tc.tile_pool(name="ssb", bufs=6) as ssbp, \
             tc.tile_pool(name="pexp", bufs=14) as pexp, \
             tc.tile_pool(name="rsu", bufs=4) as rsup, \
             tc.tile_pool(name="osb", bufs=4) as osbp, \
             tc.tile_pool(name="pA", bufs=4, space="PSUM") as pA, \
             tc.tile_pool(name="pB", bufs=2, space="PSUM") as pB, \
             tc.tile_pool(name="pC", bufs=2, space="PSUM") as pC:

            qkvT = pp.tile([128, FB * T], fp16, name="qkvT", tag="qkvT")
            oT = pp.tile([128, 4 * T], fp16, name="oT", tag="oT")
            v_all = pp.tile([128, T], fp16, name="v_all", tag="v_all")
            wt = pp.tile([128, FB * 2048], fp16, name="wt", tag="wt")
            at = pp.tile([128, 20 * 512], fp16, name="at", tag="at")
            pt = pp.tile([128, 4 * T], fp16, name="pt", tag="pt")
            bqcb = pp.tile([128, 54], f32, name="bqcb", tag="bqcb")
            kn = pp.tile([128, 258], fp16, name="kn", tag="kn")

            xts = [xtp.tile([128, 16 * 512], fp16, name=f"xt{tb}", tag="xt")
                   for tb in range(4)]

            # ---- DMA issue order: startup-critical chunks first.  The first
            # stage-1 chain reads wt[fb0] and xt[tb0] in dt order, so those
            # lead, split small so the first matmul can start ASAP.
            nc.sync.dma_start(wt[:, 0:512], wt_d[:, 0:512])
            nc.sync.dma_start(xts[0][:, 0:1024], xt_d[:, 0:1024])
            nc.sync.dma_start(wt[:, 512:1024], wt_d[:, 512:1024])
            nc.sync.dma_start(xts[0][:, 1024:2048], xt_d[:, 1024:2048])
            nc.sync.dma_start(wt[:, 1024:2048], wt_d[:, 1024:2048])
            nc.sync.dma_start(xts[0][:, 2048:4096], xt_d[:, 2048:4096])
            nc.sync.dma_start(bqcb, bqcb_d)
            nc.sync.dma_start(wt[:, 2048:4096], wt_d[:, 2048:4096])
            nc.sync.dma_start(xts[0][:, 4096:6144], xt_d[:, 4096:6144])
            nc.sync.dma_start(wt[:, 4096:6144], wt_d[:, 4096:6144])
            nc.sync.dma_start(xts[0][:, 6144:8192], xt_d[:, 6144:8192])
            for fb in range(3, FB):
                nc.sync.dma_start(wt[:, fb * 2048:(fb + 1) * 2048],
                                  wt_d[:, fb * 2048:(fb + 1) * 2048])
            nc.sync.dma_start(kn, kn_d)
            for q in range(2):
                nc.sync.dma_start(at[:, q * 5120:(q + 1) * 5120],
                                  at_d[:, q * 5120:(q + 1) * 5120])
            nc.sync.dma_start(xts[1], xt_d[:, 8192:16384])
            for q in range(2):
                nc.sync.dma_start(pt[:, q * 4096:(q + 1) * 4096],
                                  pt_d[:, q * 4096:(q + 1) * 4096])
            nc.sync.dma_start(xts[2], xt_d[:, 16384:24576])
            nc.sync.dma_start(xts[3], xt_d[:, 24576:32768])

            kT = qkvT[:, 4 * T:5 * T]
            vT = qkvT[:, 5 * T:6 * T]
            ones_col = kn[:, 0:1]          # [128, 1] for denominator matmul
            ones_row = kn[0:1, 1:129]      # [1, 128]  for broadcast matmul
            ident = kn[:, 130:258]         # [128, 128] for transposes

            # ---------------- emission helpers ----------------

            def s1_chain(tb, fb):
                """One stage-1 output tile: qkvT[fb, tb-cols]."""
                acc = pA.tile([128, 512], f32, name="acc", tag="pa")
                for dt_ in range(16):
                    nc.tensor.matmul(
                        acc,
                        wt[:, fb * 2048 + dt_ * 128: fb * 2048 + dt_ * 128 + 128],
                        xts[tb][:, dt_ * 512:(dt_ + 1) * 512],
                        start=(dt_ == 0), stop=(dt_ == 15))
                with nc.allow_low_precision(reason="qkv fp16"):
                    nc.scalar.activation(
                        qkvT[:, fb * T + tb * 512: fb * T + tb * 512 + 512],
                        acc, Ident, bias=bqcb[:, fb:fb + 1], scale=1.0)

            def s3_chain(tc_, ob):
                """One stage-3 output tile out[tc_*128:+128, ob*512:+512]."""
                acc3 = pC.tile([128, 512], f32, name="acc3", tag="pc")
                for h in range(4):
                    nc.tensor.matmul(
                        acc3,
                        oT[:, h * T + tc_ * 128: h * T + tc_ * 128 + 128],
                        pt[:, h * T + ob * 512: h * T + ob * 512 + 512],
                        start=(h == 0), stop=(h == 3))
                osb = osbp.tile([128, 512], f32, name="osb", tag="osb")
                nc.scalar.copy(osb, acc3)
                nc.sync.dma_start(
                    out_d[tc_ * 128:(tc_ + 1) * 128,
                          ob * 512:(ob + 1) * 512], osb)

            def transpose_v(jt):
                pv = pA.tile([128, 128], fp16, name="pv", tag="pa")
                nc.tensor.transpose(pv, vT[:, jt * 128:(jt + 1) * 128], ident)
                with nc.allow_low_precision(reason="v fp16"):
                    nc.vector.tensor_copy(
                        v_all[:, jt * 128:(jt + 1) * 128], pv)

            # ---------------- attention for one tb ----------------

            def attn_tb(ib, fillers):
                njb = 4 * (ib + 1)
                n_rounds = 2 * njb          # two head-pairs
                n_reserve = min(2, len(fillers))
                n_loop = len(fillers) - n_reserve
                st = {}                     # h -> dict of live state
                pend = []                   # (h, jb, psb, c0) awaiting AV
                post = []                   # deferred flush closures
                fidx = 0
                rnd = 0
                credit = 1.0

                def filler():
                    nonlocal fidx
                    if fidx < len(fillers):
                        fillers[fidx]()
                        fidx += 1

                def flush_gsum(h):
                    # fold queued exp tiles into the running denominator sum
                    # (DVE, all-fp16 SBUF operands -> 2x mode), in place into
                    # one accumulator tile.  Called one round after the exp
                    # is issued so DVE never waits on ACT.
                    s = st[h]
                    for psb, c0 in s["gq"]:
                        with nc.allow_low_precision(reason="denom fp16"):
                            if s["gsum"] is None:
                                g = pexp.tile([128, 512], fp16, name="gsum",
                                              tag="psb")
                                nc.vector.tensor_copy(g, psb)
                                s["gsum"] = g
                            else:
                                g = s["gsum"]
                                nc.vector.tensor_tensor(
                                    g[:, c0:512], g[:, c0:512],
                                    psb[:, c0:512], add)
                    s["gq"] = []

                def emit_scores(hpair, jb):
                    for h in hpair:
                        s = st[h]
                        flush_gsum(h)
                        dd = jb - 4 * ib
                        c0 = 128 * dd if dd > 0 else 0
                        i0 = ib * 512 + c0
                        spsum = pA.tile([128, 512], f32, name="spsum",
                                        tag="pa")
                        nc.tensor.matmul(
                            spsum[:, c0:512], kT[:, jb * 128:(jb + 1) * 128],
                            qkvT[:, h * T + i0: h * T + (ib + 1) * 512],
                            start=True, stop=True)
                        ssb = ssbp.tile([128, 512], f32, name="ssb", tag="ssb")
                        if dd >= 0:
                            nc.vector.tensor_tensor(
                                ssb[:, c0:512], spsum[:, c0:512],
                                at[:, (4 + h * 4 + dd) * 512 + c0:
                                      (5 + h * 4 + dd) * 512], add)
                            bias = 0.0
                        else:
                            nc.vector.tensor_tensor(
                                ssb, spsum, at[:, h * 512:(h + 1) * 512], add)
                            k_ = 4 * ib - jb
                            bias = bqcb[:, 6 + h * 12 + k_ - 1:
                                        6 + h * 12 + k_]
                        psb = pexp.tile([128, 512], fp16, name="psb",
                                        tag="psb")
                        with nc.allow_low_precision(reason="softmax fp16"):
                            nc.scalar.activation(psb[:, c0:512],
                                                 ssb[:, c0:512], Exp,
                                                 bias=bias, scale=1.0)
                        s["gq"].append((psb, c0))
                        pend.append((h, jb, psb, c0))

                def emit_av():
                    h, jb, psb, c0 = pend.pop(0)
                    s = st[h]
                    nc.tensor.matmul(
                        s["opsum"][:, c0:512],
                        v_all[:, jb * 128:(jb + 1) * 128],
                        psb[:, c0:512],
                        start=(jb == 0), stop=(jb == njb - 1),
                        skip_group_check=True)

                def emit_dred(h):
                    s = st[h]
                    flush_gsum(h)
                    dsc = pA.tile([128, 512], f32, name="dsc", tag="pa")
                    nc.tensor.matmul(dsc[0:1, :], ones_col, s["gsum"],
                                     start=True, stop=True)
                    s["dsc"] = dsc
                    rsum = rsup.tile([1, 512], fp16, name="rsum", tag="rsum")
                    with nc.allow_low_precision(reason="softmax recip fp16"):
                        nc.vector.reciprocal(rsum, dsc[0:1, :])
                    s["rsum"] = rsum

                def emit_norm(h):
                    s = st[h]
                    rps = pA.tile([128, 512], f32, name="rps", tag="pa")
                    nc.tensor.matmul(rps, ones_row, s["rsum"],
                                     start=True, stop=True)
                    rsb = ssbp.tile([128, 512], f32, name="rsb", tag="ssb")
                    nc.scalar.copy(rsb, rps)
                    with nc.allow_low_precision(reason="oT fp16"):
                        nc.vector.tensor_tensor(
                            oT[:, h * T + ib * 512: h * T + ib * 512 + 512],
                            s["opsum"], rsb, mult)

                for hp in (0, 2):
                    hpair = (hp, hp + 1)
                    for h in hpair:
                        st[h] = {
                            "opsum": pB.tile([128, 512], f32, name="opsum",
                                             tag="pb"),
                            "gsum": None, "gq": [], "rsum": None, "dsc": None,
                        }
                    for jb in range(njb):
                        emit_scores(hpair, jb)
                        # run deferred flush closures from the previous pair
                        while post and rnd >= post[0][0]:
                            post.pop(0)[1]()
                        while len(pend) > 2 * AV_LAG:
                            emit_av()
                        rnd += 1
                        credit += n_loop / n_rounds
                        while credit >= 1.0 and fidx < n_loop:
                            filler()
                            credit -= 1.0
                    # chain end: drain AVs for this pair (next pair's scores
                    # interleave via the pend window on the following rounds)
                    if hp == 0:
                        # keep remaining AVs pending into next pair's rounds;
                        # defer dred/norm past 1 and 3 more rounds
                        post.append((rnd + 1, (lambda a=hp: emit_dred(a))))
                        post.append((rnd + 1, (lambda a=hp + 1: emit_dred(a))))
                        post.append((rnd + 3, (lambda a=hp: emit_norm(a))))
                        post.append((rnd + 3, (lambda a=hp + 1:
                                               emit_norm(a))))
                    else:
                        while pend:
                            emit_av()
                        emit_dred(hp)
                        filler()
                        emit_dred(hp + 1)
                        filler()
                        emit_norm(hp)
                        emit_norm(hp + 1)
                # leftover fillers
                while fidx < len(fillers):
                    filler()

            # ---------------- main schedule ----------------

            for fb in range(FB):
                s1_chain(0, fb)

            # stage-3 tiles are spread as PE filler over later blocks so each
            # attention block gets work in proportion to its round count
            s3_sched = {0: [], 1: [], 2: [], 3: []}
            for tb_src in range(3):
                items = [(tc_, ob)
                         for tc_ in range(4 * tb_src, 4 * tb_src + 4)
                         for ob in range(4)]
                if tb_src == 0:                  # tb1 keeps only s1 filler
                    s3_sched[2] += items
                else:                            # tb2/tb3 split the rest
                    s3_sched[3] += items

            for tb in range(4):
                for jt in range(4 * tb, 4 * tb + 4):
                    transpose_v(jt)
                fillers = []
                if tb < 3:                       # stage-1 for tb+1
                    fillers += [
                        (lambda t=tb + 1, f=fb_: s1_chain(t, f))
                        for fb_ in range(FB)]
                fillers += [(lambda c=tc_, o=ob: s3_chain(c, o))
                            for (tc_, ob) in s3_sched[tb]]
                attn_tb(tb, fillers)

            for tc_ in range(12, 16):            # final stage-3 slice
                for ob in range(4):
                    s3_chain(tc_, ob)

    nc.compile()
    return nc


def get_nc():
    if "nc" not in _CACHE:
        _CACHE["nc"] = _build_nc()
    return _CACHE["nc"]


# --------------------------------------------------------------------------
# host-side packing
# --------------------------------------------------------------------------

def _expected_slopes():
    return 2.0 ** (-8.0 * (np.arange(1, NH + 1) / NH))  # float64


def _check_structure(attn_mask, alibi_bias):
    """Return exact float64 alibi slopes if inputs match the expected
    causal-mask + rank-1 alibi structure, else None."""
    am = np.asarray(attn_mask)
    if am.shape != (1, 1, T, T):
        return None
    if not np.array_equal(am[0, 0], np.tril(np.ones((T, T), dtype=bool))):
        return None
    al = np.asarray(alibi_bias, dtype=np.float32)
    if al.shape != (1, NH, T, T):
        return None
    slopes = _expected_slopes()
    if not np.allclose(al[0, :, 0, 1], slopes.astype(np.float32),
                       rtol=1e-6, atol=1e-8):
        return None
    idx = np.arange(T, dtype=np.float64)
    rel = idx[None, :] - idx[:, None]
    for h in range(NH):
        ref = (slopes[h] * rel).astype(np.float32)
        if not np.array_equal(al[0, h], ref):
            if not np.allclose(al[0, h], ref, rtol=1e-5, atol=1e-4):
                return None
    return slopes


def _pack_core_inputs(x, qkv_w, qkv_b, proj_w, slopes):
    f16 = np.float16
    x = np.asarray(x, dtype=np.float32)
    qkv_w = np.asarray(qkv_w, dtype=np.float32)
    qkv_b = np.asarray(qkv_b, dtype=np.float32)
    proj_w = np.asarray(proj_w, dtype=np.float32)
    inv = np.float32(1.0 / math.sqrt(HD))

    xts = []
    for b in range(B):
        a = x[b].T.reshape(16, 128, 4, 512)          # [dt, p, tb, tl]
        xts.append(np.ascontiguousarray(
            a.transpose(1, 2, 0, 3).reshape(128, 16 * T).astype(f16)))

    per_g = []
    jj = np.arange(128, dtype=np.float64)[:, None]
    ii = np.arange(512, dtype=np.float64)[None, :]
    for g in range(KVH):
        Wq = qkv_w[512 * g:512 * (g + 1)] * inv
        Wk = qkv_w[D + 128 * g: D + 128 * (g + 1)]
        Wv = qkv_w[D + 512 + 128 * g: D + 512 + 128 * (g + 1)]
        Wc = np.concatenate([Wq, Wk, Wv], axis=0)    # [768, 2048]
        wt = np.ascontiguousarray(
            Wc.reshape(6, 128, 16, 128).transpose(3, 0, 2, 1)
            .reshape(128, FB * 2048).astype(f16))
        bc = np.concatenate([qkv_b[512 * g:512 * (g + 1)] * inv,
                             qkv_b[D + 128 * g: D + 128 * (g + 1)],
                             qkv_b[D + 512 + 128 * g: D + 512 + 128 * (g + 1)]])
        bqcb = np.zeros((128, 54), dtype=np.float32)
        bqcb[:, 0:6] = bc.reshape(FB, 128).T

        at = np.empty((128, 20 * 512), dtype=f16)
        for h in range(GRP):
            s = slopes[4 * g + h]
            at[:, h * 512:(h + 1) * 512] = (s * (jj - ii)).astype(f16)
            for dd in range(4):
                A = (s * (jj + 128 * dd - ii)).astype(f16)
                A[(jj + 128 * dd - ii) > 0] = f16(NEG16)
                at[:, (4 + h * 4 + dd) * 512:(5 + h * 4 + dd) * 512] = A
            for k_ in range(1, 13):
                bqcb[:, 6 + h * 12 + k_ - 1] = np.float32(s * (-128.0 * k_))

        ptp = np.ascontiguousarray(
            proj_w[:, 512 * g:512 * (g + 1)].T
            .reshape(4, 128, T).transpose(1, 0, 2).reshape(128, 4 * T)
            .astype(f16))
        per_g.append({"wt": wt, "bqcb": bqcb, "at": at, "pt": ptp})

    kn = np.zeros((128, 258), dtype=f16)
    kn[:, 0] = 1.0
    kn[0, 1:129] = 1.0
    kn[:, 130:258] = np.eye(128, dtype=np.float32)

    in_maps = []
    for c in range(NCORE):
        b, g = divmod(c, KVH)
        m = dict(per_g[g])
        m["xt"] = xts[b]
        m["kn"] = kn
        in_maps.append(m)
    return in_maps


# --------------------------------------------------------------------------
# numpy fallback (only used if inputs don't match the expected structure)
# --------------------------------------------------------------------------

def _numpy_reference(x, attn_mask, alibi_bias, qkv_w, qkv_b, proj_w, proj_b):
    x = np.asarray(x, dtype=np.float32)
    b, t, c = x.shape
    qkv = x @ qkv_w.T + qkv_b
    q = qkv[..., :D].reshape(b, t, KVH, GRP, HD).transpose(0, 2, 3, 1, 4)
    k = qkv[..., D:D + 512].reshape(b, t, KVH, HD).transpose(0, 2, 1, 3)
    v = qkv[..., D + 512:].reshape(b, t, KVH, HD).transpose(0, 2, 1, 3)
    scale = 1.0 / math.sqrt(HD)
    att = np.einsum("bkgtd,bksd->bkgts", q, k).astype(np.float32) * scale
    att = att + np.asarray(alibi_bias).reshape(1, KVH, GRP, t, t)
    mask = np.asarray(attn_mask)[:, :, None]
    att = np.where(mask, att, -np.inf)
    att = att - att.max(axis=-1, keepdims=True)
    np.exp(att, out=att)
    att /= att.sum(axis=-1, keepdims=True)
    out = np.einsum("bkgts,bksd->bkgtd", att, v)
    out = out.transpose(0, 3, 1, 2, 4).reshape(b, t, c)
    return (out @ proj_w.T + proj_b).astype(np.float32)


# --------------------------------------------------------------------------
# entry point
# --------------------------------------------------------------------------

def kernel(x, attn_mask, alibi_bias, qkv_w, qkv_b, proj_w, proj_b):
    from concourse import bass_utils

    slopes = _check_structure(attn_mask, alibi_bias)
    if slopes is None:
        return _numpy_reference(x, attn_mask, alibi_bias, qkv_w, qkv_b,
                                proj_w, proj_b)

    nc = get_nc()
    in_maps = _pack_core_inputs(x, qkv_w, qkv_b, proj_w, slopes)
    res = bass_utils.run_bass_kernel_spmd(nc, in_maps,
                                          core_ids=list(range(NCORE)))
    proj_b = np.asarray(proj_b, dtype=np.float32)
    out = np.empty((B, T, D), dtype=np.float32)
    for b in range(B):
        acc = res.results[4 * b + 0]["out"].astype(np.float32, copy=True)
        for g in range(1, KVH):
            acc += res.results[4 * b + g]["out"]
        out[b] = acc + proj_b
    return out
